# revision 1
# baseline (speedup 1.0000x reference)
"""Trainium2 Bass kernel for EnergyConstrainedPredictiveCodingModel.

Data-parallel over the batch dim across 8 NeuronCores; weights replicated.
Natural (rows-on-partitions) activation layout; activations entering a
matmul are transposed on the PE and rounded to float32r at the PSUM->SBUF
evict.  All model matmuls run as float32r (full-rate streaming for N>=256,
~1.6e-4 relative rounding vs fp32).

Model (per reference):
  B=8192, D=1024, L=512, H=512, REC=256, MAX_NORM=0.5
  out = concat([z, h_new, h2_new, sigma_p, theta, sst_inh, theta_ff,
                z_energy, I_hat, layer_1_error, layer_2_error], -1)
"""

import numpy as np
from contextlib import ExitStack

import concourse.bass as bass
import concourse.mybir as mybir
import concourse.tile as tile
from concourse import bacc
from concourse.bass_utils import run_bass_kernel_spmd
from concourse.masks import make_identity

B, D, L, H, REC = 8192, 1024, 512, 512, 256
MAX_NORM = 0.5
N_CORES = 8
BL = B // N_CORES            # rows per core
P = 128                      # partitions
NT = BL // P                 # row tiles per core
OUT_W = 9 * L + 2 * D        # 6656

F32 = mybir.dt.float32
F32R = mybir.dt.float32r
AF = mybir.ActivationFunctionType
OP = mybir.AluOpType

# output column offsets
OFF_Z = 0
OFF_HN = L
OFF_H2N = 2 * L
OFF_SP = 3 * L
OFF_TH = 4 * L
OFF_SST = 5 * L
OFF_TFF = 6 * L
OFF_ZE = 7 * L
OFF_IH = 8 * L
OFF_L1 = 8 * L + D
OFF_L2 = 8 * L + 2 * D


def _load_weight(nc, pool, dram_ap, K, N, name, dtype=F32R):
    """DRAM [K, N] -> SBUF [128, K//128, N] (chunked along contraction)."""
    t = pool.tile([P, K // P, N], dtype, tag=name)
    nc.sync.dma_start(out=t, in_=dram_ap.rearrange("(c p) n -> p c n", p=P))
    return t


def _mm_group(nc, out_ps, lhsT_sb, w_sb, nk, first=True, last=True, n_slice=None):
    """Accumulate out_ps += lhsT.T @ w over nk 128-chunks (f32r operands)."""
    for c in range(nk):
        rhs = w_sb[:, c, :] if n_slice is None else w_sb[:, c, n_slice]
        nc.tensor.matmul(
            out_ps,
            lhsT_sb[:, c, :],
            rhs,
            start=(first and c == 0),
            stop=(last and c == nk - 1),
        )


def _act_recip(nc, out, in_):
    eng = nc.scalar
    return eng.add_instruction(
        mybir.InstActivation(
            name=nc.get_next_instruction_name(),
            func=AF.Reciprocal,
            ins=[
                eng.lower_ap(in_),
                mybir.ImmediateValue(dtype=F32, value=0.0),
                mybir.ImmediateValue(dtype=F32, value=1.0),
                mybir.ImmediateValue(dtype=F32, value=0.0),
            ],
            outs=[eng.lower_ap(out)],
        )
    )


def _build_program(bl=BL):
    nc = bacc.Bacc(trn_type="TRN2", target_bir_lowering=False, debug=False)
    nt = bl // P

    def din(name, shape, dtype=F32):
        return nc.dram_tensor(name, shape, dtype, kind="ExternalInput").ap()

    it_d = din("it", [bl, D])
    h_d = din("h", [bl, H])
    h2_d = din("h2", [bl, H])
    spp_d = din("spp", [bl, L])
    tffp_d = din("tffp", [bl, L])
    tp_d = din("tp", [bl, L])
    sstp_d = din("sstp", [bl, L])
    epsz_d = din("epsz", [bl, L])
    epszh_d = din("epszh", [bl, L])
    # weights, pre-transposed on host to [in, out] except wrec1 (natural)
    wpm_d = din("wpm_t", [D, L], F32R)
    wps_d = din("wps_t", [D, L], F32R)
    wzh_d = din("wzh_t", [L, H], F32R)
    whh_d = din("whh_t", [H, H])
    wh2h2_d = din("wh2h2_t", [H, H], F32R)
    wzh2_d = din("wzh2_t", [L, H], F32R)
    wprm_d = din("wprm_t", [H, L], F32R)
    wprs_d = din("wprs_t", [H, L], F32R)
    wvip_d = din("wvip_t", [L, L], F32R)
    wt2z_d = din("wt2z_t", [L, L], F32R)
    wi2t_d = din("wi2t_t", [D, L], F32R)
    wrec1_d = din("wrec1", [REC, L], F32R)
    wrec2_d = din("wrec2_t", [REC, D], F32R)
    bps_d = din("bps", [1, L])

    out_d = nc.dram_tensor("out", [bl, OUT_W], F32, kind="ExternalOutput").ap()

    with tile.TileContext(nc) as tc, ExitStack() as ctx:
        weights = ctx.enter_context(tc.tile_pool(name="weights", bufs=1))
        consts = ctx.enter_context(tc.tile_pool(name="consts", bufs=1))
        psum = ctx.enter_context(tc.tile_pool(name="psum", bufs=5, space="PSUM"))
        pool_in = ctx.enter_context(tc.tile_pool(name="inp", bufs=2))
        pool_in1 = ctx.enter_context(tc.tile_pool(name="inp1", bufs=1))
        pool_tr = ctx.enter_context(tc.tile_pool(name="trans", bufs=1))
        pool_tr2 = ctx.enter_context(tc.tile_pool(name="trans2", bufs=2))

        ident = consts.tile([P, P], F32)
        make_identity(nc, ident)
        ones_row_f = consts.tile([1, P], F32)
        nc.vector.memset(ones_row_f, 1.0)
        ones_row = consts.tile([1, P], F32R)
        nc.scalar.copy(ones_row, ones_row_f)
        ones_col = consts.tile([P, 1], F32)
        nc.vector.memset(ones_col, 1.0)
        neg1_col = consts.tile([P, 1], F32)
        nc.vector.memset(neg1_col, -1.0)
        bps = consts.tile([1, L], F32R)

        def load_inputs(t, it_tile=None):
            rows = slice(t * P, (t + 1) * P)
            d = {}
            if it_tile is not None:
                d["it"] = it_tile
            else:
                d["it"] = pool_in.tile([P, D], F32, tag="it", name="it_sb", bufs=3)
                nc.sync.dma_start(out=d["it"], in_=it_d[rows, :])
            d["h"] = pool_in1.tile([P, H], F32, tag="h", name="h_sb")
            nc.sync.dma_start(out=d["h"], in_=h_d[rows, :])
            d["h2"] = pool_in1.tile([P, H], F32, tag="h2", name="h2_sb")
            nc.sync.dma_start(out=d["h2"], in_=h2_d[rows, :])
            d["tffp"] = pool_in1.tile([P, L], F32, tag="tffp", name="tffp_sb")
            nc.sync.dma_start(out=d["tffp"], in_=tffp_d[rows, :])
            d["spp"] = pool_in1.tile([P, L], F32, tag="spp", name="spp_sb")
            nc.sync.dma_start(out=d["spp"], in_=spp_d[rows, :])
            d["tp"] = pool_in1.tile([P, L], F32, tag="tp", name="tp_sb")
            nc.sync.dma_start(out=d["tp"], in_=tp_d[rows, :])
            d["sstp"] = pool_in1.tile([P, L], F32, tag="sstp", name="sstp_sb")
            nc.sync.dma_start(out=d["sstp"], in_=sstp_d[rows, :])
            d["epsz"] = pool_in1.tile([P, L], F32, tag="epsz", name="epsz_sb")
            nc.sync.dma_start(out=d["epsz"], in_=epsz_d[rows, :])
            d["epszh"] = pool_in.tile([P, L], F32, tag="epszh", name="epszh_sb")
            nc.sync.dma_start(out=d["epszh"], in_=epszh_d[rows, :])
            return d

        # PE transpose src [128, nblk*128] -> dst [128, nblk, 128]; the
        # transpose runs in plain f32, the PSUM->SBUF evict rounds to f32r
        def transpose_in(dst, src, nblk):
            g = 0
            while g * 4 < nblk:
                k = min(4, nblk - g * 4)
                ps = psum.tile([P, 512], F32, tag="ps")
                for j in range(k):
                    blk = g * 4 + j
                    nc.tensor.transpose(
                        ps[:, j * P:(j + 1) * P],
                        src[:, blk * P:(blk + 1) * P],
                        ident,
                    )
                dslice = dst[:, g * 4:g * 4 + k, :].rearrange("p c n -> p (c n)")
                nc.scalar.copy(dslice, ps[:, : k * P])
                g += 1

        def make_trans(t, d):
            tt = {}
            tt["itT"] = pool_tr.tile([P, D // P, P], F32R, tag="itT", name="itT")
            transpose_in(tt["itT"], d["it"], D // P)
            tt["hT"] = pool_tr2.tile([P, H // P, P], F32R, tag="hT", name="hT")
            transpose_in(tt["hT"], d["h"], H // P)
            tt["h2T"] = pool_tr2.tile([P, H // P, P], F32R, tag="h2T", name="h2T")
            transpose_in(tt["h2T"], d["h2"], H // P)
            return tt

        # ---- prologue: first row-tile's inputs + transposes before weights ----
        pre_in = load_inputs(0)
        pre_tr = make_trans(0, pre_in)

        # ---- setup-feeding weight DMAs + parametrizations ----
        whh = weights.tile([P, H // P, H], F32R, tag="whh")
        wvip = weights.tile([P, L // P, L], F32R, tag="wvip")
        wt2z = weights.tile([P, L // P, L], F32R, tag="wt2z")
        wrec = weights.tile([P, L // P, D], F32R, tag="wrec")

        with tc.tile_pool(name="setup", bufs=1) as setup:
            # b_prior_sigma: relu + round to f32r
            bps_st = setup.tile([1, L], F32, tag="bps_st")
            nc.sync.dma_start(out=bps_st, in_=bps_d)
            nc.scalar.activation(bps, bps_st, AF.Relu)

            # W_h_to_h spectral clip: W * min(1, MAX_NORM / ||W||_F)
            whh_st = setup.tile([P, H // P, H], F32, tag="stage_a")
            nc.sync.dma_start(
                out=whh_st, in_=whh_d.rearrange("(c p) n -> p c n", p=P)
            )
            whh_f = whh_st.rearrange("p c n -> p (c n)")
            nchk = (H // P) * H // 512
            acc = setup.tile([P, nchk], F32)
            for j in range(nchk):
                scr = setup.tile([P, 512], F32, tag="ttr_scr")
                chunk = whh_f[:, j * 512:(j + 1) * 512]
                nc.scalar.activation(
                    scr, chunk, AF.Square, accum_out=acc[:, j:j + 1]
                )
            sq_sum = setup.tile([P, 1], F32)
            nc.vector.tensor_reduce(sq_sum, acc, mybir.AxisListType.X, OP.add)
            nrm2_ps = psum.tile([1, 1], F32, tag="ps", name="nrm2_ps")
            nc.tensor.matmul(nrm2_ps, sq_sum, ones_col, start=True, stop=True)
            nrm = setup.tile([1, 1], F32)
            nc.scalar.activation(nrm, nrm2_ps, AF.Sqrt)
            rn = setup.tile([1, 1], F32)
            nc.vector.reciprocal(rn, nrm)
            scale = setup.tile([1, 1], F32)
            nc.vector.tensor_scalar(scale, rn, MAX_NORM, 1.0, OP.mult, OP.min)
            scale_ps = psum.tile([P, 1], F32, tag="ps", name="scale_ps")
            nc.tensor.matmul(scale_ps, ones_row_f, scale, start=True, stop=True)
            scale_bc = setup.tile([P, 1], F32)
            nc.scalar.copy(scale_bc, scale_ps)
            nc.vector.tensor_scalar(whh_f, whh_f, scale_bc, None, OP.mult)
            nc.scalar.activation(
                whh.rearrange("p c n -> p (c n)"), whh_f, AF.Identity
            )

            # fuse W_rec = (W_rec2 @ W_rec1).T = W_rec1.T @ W_rec2.T
            wrec1 = _load_weight(nc, setup, wrec1_d, REC, L, "wrec1")
            wrec2 = _load_weight(nc, setup, wrec2_d, REC, D, "stage_a")
            for m in range(L // P):
                for half in range(2):
                    ps = psum.tile([P, 512], F32, tag="ps")
                    for c in range(REC // P):
                        nc.tensor.matmul(
                            ps,
                            wrec1[:, c, m * P:(m + 1) * P],
                            wrec2[:, c, half * 512:(half + 1) * 512],
                            start=(c == 0),
                            stop=(c == REC // P - 1),
                        )
                    nc.scalar.copy(wrec[:, m, half * 512:(half + 1) * 512], ps)

            # ---- stage-1 weights (ordered by first use in the pipeline) ----
            def relu_weight(wdst, wsrc_d):
                nc.sync.dma_start(
                    out=wdst, in_=wsrc_d.rearrange("(c p) n -> p c n", p=P)
                )
                nc.scalar.activation(
                    wdst.rearrange("p c n -> p (c n)"),
                    wdst.rearrange("p c n -> p (c n)").bitcast(F32),
                    AF.Relu,
                )

            # ordered to match the PE stream's first-use order
            wprs = _load_weight(nc, weights, wprs_d, H, L, "wprs")
            wi2t = _load_weight(nc, weights, wi2t_d, D, L, "wi2t")
            relu_weight(wvip, wvip_d)
            pre_in1 = load_inputs(1)
            it2_pre = pool_in.tile([P, D], F32, tag="it", name="it_sb", bufs=3)
            nc.sync.dma_start(out=it2_pre, in_=it_d[2 * P:3 * P, :])
            wprm = _load_weight(nc, weights, wprm_d, H, L, "wprm")
            wpm = _load_weight(nc, weights, wpm_d, D, L, "wpm")
            wps = _load_weight(nc, weights, wps_d, D, L, "wps")
            relu_weight(wt2z, wt2z_d)
            wzh = _load_weight(nc, weights, wzh_d, L, H, "wzh")
            wh2h2 = _load_weight(nc, weights, wh2h2_d, H, H, "wh2h2")
            wzh2 = _load_weight(nc, weights, wzh2_d, L, H, "wzh2")

        # remaining per-iteration pools (reuse setup's released space)
        pool_im = ctx.enter_context(tc.tile_pool(name="interm", bufs=1))
        pool_out = ctx.enter_context(tc.tile_pool(name="outs", bufs=1))
        pool_out2 = ctx.enter_context(tc.tile_pool(name="outs2", bufs=2))

        # ---- software-pipelined main loop ----
        # stage1(t) = input transposes + all matmuls/elementwise through theta
        # tail(t)   = theta-transpose onward (sst, z, h_new, I_hat, errors)
        # Emission order: S1(0), S1(1), tail(0), S1(2), tail(1), ... so the PE
        # always has iteration t+1's independent matmuls queued while t's
        # serial theta chain (incl. the ~3.3us reciprocal) runs on DVE.
        # PSUM: "ps" = transient ring (5 banks); "psh" = mup/muq/sq held
        # from stage1 until their tail evictions (3 banks).

        def stage1(t, d, tt):
            rows = slice(t * P, (t + 1) * P)
            st = {"d": d, "tt": tt, "rows": rows}
            hT, h2T, itT = tt["hT"], tt["h2T"], tt["itT"]

            # matmuls whose consumers are inside stage1 come first
            sigp_ps = psum.tile([P, L], F32, tag="ps", name="sigp_ps")
            nc.tensor.matmul(sigp_ps, ones_row, bps, start=True, stop=False)
            _mm_group(nc, sigp_ps, hT, wprs, H // P, first=False)
            ith_ps = psum.tile([P, L], F32, tag="ps", name="ith_ps")
            _mm_group(nc, ith_ps, itT, wi2t, D // P)

            # sigma_p = 0.8*relu(h@Wps.T + b) + 0.2*spp
            sigp_sb = pool_out2.tile([P, L], F32, tag="sigp", name="sigp_sb")
            nc.scalar.activation(sigp_sb, sigp_ps, AF.Relu, scale=0.8)
            nc.vector.scalar_tensor_tensor(
                sigp_sb, d["spp"], 0.2, sigp_sb, OP.mult, OP.add
            )
            nc.sync.dma_start(out=out_d[rows, OFF_SP:OFF_SP + L], in_=sigp_sb)
            st["sigp"] = sigp_sb

            # theta_ff = tanh(0.4*tffp + exp(-50|tffp|)*(I@Wi2t.T))^2
            a1_sb = pool_im.tile([P, L], F32, tag="scr1", name="a1_sb")
            nc.scalar.activation(a1_sb, d["tffp"], AF.Abs)
            nc.scalar.activation(a1_sb, a1_sb, AF.Exp, scale=-50.0)
            tff_sb = pool_out.tile([P, L], F32, tag="tff", name="tff_sb")
            nc.vector.tensor_mul(tff_sb, a1_sb, ith_ps)
            nc.vector.scalar_tensor_tensor(
                tff_sb, d["tffp"], 0.4, tff_sb, OP.mult, OP.add
            )
            nc.scalar.activation(tff_sb, tff_sb, AF.Tanh)
            nc.scalar.activation(tff_sb, tff_sb, AF.Square)
            nc.sync.dma_start(out=out_d[rows, OFF_TFF:OFF_TFF + L], in_=tff_sb)

            # vip chain: theta = 0.1*tp + tff/(1 + sigma_p@Wvip_p.T)
            sigpT = pool_tr.tile([P, L // P, P], F32R, tag="sigpT", name="sigpT")
            transpose_in(sigpT, sigp_sb, L // P)
            vip_ps = psum.tile([P, L], F32, tag="ps", name="vip_ps")
            _mm_group(nc, vip_ps, sigpT, wvip, L // P)

            # matmuls consumed only by the tail go last (their PSUM is held)
            mup_ps = psum.tile([P, L], F32, tag="psh", name="mup_ps", bufs=3)
            _mm_group(nc, mup_ps, h2T, wprm, H // P)
            muq_ps = psum.tile([P, L], F32, tag="psh", name="muq_ps", bufs=3)
            _mm_group(nc, muq_ps, itT, wpm, D // P)
            sq_ps = psum.tile([P, L], F32, tag="psh", name="sq_ps", bufs=3)
            _mm_group(nc, sq_ps, itT, wps, D // P)
            st["mup_ps"], st["muq_ps"], st["sq_ps"] = mup_ps, muq_ps, sq_ps

            theta_sb = pool_out2.tile([P, L], F32, tag="theta", name="theta_sb")
            nc.vector.tensor_scalar_add(theta_sb, vip_ps, 1.0)
            _act_recip(nc, theta_sb, theta_sb)
            nc.vector.tensor_mul(theta_sb, tff_sb, theta_sb)
            nc.vector.scalar_tensor_tensor(
                theta_sb, d["tp"], 0.1, theta_sb, OP.mult, OP.add
            )
            nc.sync.dma_start(out=out_d[rows, OFF_TH:OFF_TH + L], in_=theta_sb)
            st["theta"] = theta_sb
            return st

        def tail(t, st):
            rows = st["rows"]
            d, tt = st["d"], st["tt"]
            it_sb, hT, h2T = d["it"], tt["hT"], tt["h2T"]
            sigp_sb, theta_sb = st["sigp"], st["theta"]

            # held-PSUM evictions
            mup_sb = pool_im.tile([P, L], F32, tag="mup", name="mup_sb")
            nc.scalar.activation(mup_sb, st["mup_ps"], AF.Relu)
            muq_sb = pool_im.tile([P, L], F32, tag="scr2", name="muq_sb")
            nc.scalar.activation(muq_sb, st["muq_ps"], AF.Relu)
            s_sb = pool_im.tile([P, L], F32, tag="s", name="s_sb")
            nc.vector.tensor_scalar_max(s_sb, st["sq_ps"], 0.0)
            nc.scalar.activation(s_sb, s_sb, AF.Tanh, scale=0.005)

            # raw_z = tanh(mu_q + eps_z*(s - 0.5))  (independent of theta/sst)
            rz_sb = pool_im.tile([P, L], F32, tag="scr1", name="rz_sb")
            nc.vector.scalar_tensor_tensor(
                rz_sb, s_sb, 0.5, d["epsz"], OP.mult, OP.mult
            )
            nc.vector.tensor_add(rz_sb, rz_sb, muq_sb)
            nc.scalar.activation(rz_sb, rz_sb, AF.Tanh)

            # sst_inh = 0.8*sstp + theta@Wt2z_p.T
            thetaT = pool_tr.tile([P, L // P, P], F32R, tag="thetaT", name="thetaT")
            transpose_in(thetaT, theta_sb, L // P)
            sst_ps = psum.tile([P, L], F32, tag="ps", name="sst_ps")
            _mm_group(nc, sst_ps, thetaT, wt2z, L // P)
            sst_sb = pool_out.tile([P, L], F32, tag="sst", name="sst_sb")
            nc.vector.scalar_tensor_tensor(
                sst_sb, d["sstp"], 0.8, sst_ps, OP.mult, OP.add
            )
            nc.sync.dma_start(out=out_d[rows, OFF_SST:OFF_SST + L], in_=sst_sb)

            # z = relu(raw_z - sst)   (== z_energy)
            z_sb = pool_out.tile([P, L], F32, tag="z", name="z_sb")
            nc.vector.tensor_sub(z_sb, rz_sb, sst_sb)
            nc.vector.tensor_scalar_max(z_sb, z_sb, 0.0)
            nc.sync.dma_start(out=out_d[rows, OFF_Z:OFF_Z + L], in_=z_sb)
            nc.sync.dma_start(out=out_d[rows, OFF_ZE:OFF_ZE + L], in_=z_sb)

            # h_new / h2_new
            zT = pool_tr.tile([P, L // P, P], F32R, tag="zT", name="zT")
            transpose_in(zT, z_sb, L // P)
            hn_ps = psum.tile([P, H], F32, tag="ps", name="hn_ps")
            _mm_group(nc, hn_ps, hT, whh, H // P, last=False)
            _mm_group(nc, hn_ps, zT, wzh, L // P, first=False)
            hn_sb = pool_out.tile([P, H], F32, tag="hn", name="hn_sb")
            nc.scalar.activation(hn_sb, hn_ps, AF.Relu)
            nc.sync.dma_start(out=out_d[rows, OFF_HN:OFF_HN + H], in_=hn_sb)
            h2n_ps = psum.tile([P, H], F32, tag="ps", name="h2n_ps")
            _mm_group(nc, h2n_ps, h2T, wh2h2, H // P, last=False)
            _mm_group(nc, h2n_ps, zT, wzh2, L // P, first=False)
            h2n_sb = pool_out.tile([P, H], F32, tag="hn", name="h2n_sb")
            nc.scalar.activation(h2n_sb, h2n_ps, AF.Relu)
            nc.sync.dma_start(out=out_d[rows, OFF_H2N:OFF_H2N + H], in_=h2n_sb)

            # I_hat = sigmoid(z @ W_rec.T - 2); layer_1_error = (I_t - I_hat)^2
            for half in range(2):
                hsl = slice(half * 512, (half + 1) * 512)
                ih_ps = psum.tile([P, 512], F32, tag="ps", name="ih_ps")
                _mm_group(nc, ih_ps, zT, wrec, L // P, n_slice=hsl)
                ih_sb = pool_out.tile([P, 512], F32, tag="ih", name="ih_sb")
                nc.scalar.activation(ih_sb, ih_ps, AF.Tanh, scale=0.5, bias=neg1_col)
                nc.vector.tensor_scalar(ih_sb, ih_sb, 0.5, 0.5, OP.mult, OP.add)
                nc.sync.dma_start(
                    out=out_d[rows, OFF_IH + half * 512:OFF_IH + half * 512 + 512],
                    in_=ih_sb,
                )
                l1_sb = pool_out.tile([P, 512], F32, tag="l1", name="l1_sb")
                nc.vector.tensor_sub(l1_sb, it_sb[:, hsl], ih_sb)
                nc.vector.tensor_mul(l1_sb, l1_sb, l1_sb)
                nc.sync.dma_start(
                    out=out_d[rows, OFF_L1 + half * 512:OFF_L1 + half * 512 + 512],
                    in_=l1_sb,
                )

            # layer_2_error = (z - mu_p - eps_zhat*sigma_p)^2
            l2_sb = pool_out.tile([P, L], F32, tag="sst", name="l2_sb")
            zh1_sb = pool_im.tile([P, L], F32, tag="scr2", name="zh1_sb")
            nc.vector.tensor_mul(zh1_sb, d["epszh"], sigp_sb)
            nc.vector.tensor_sub(l2_sb, z_sb, mup_sb)
            nc.vector.tensor_sub(l2_sb, l2_sb, zh1_sb)
            nc.vector.tensor_mul(l2_sb, l2_sb, l2_sb)
            nc.sync.dma_start(out=out_d[rows, OFF_L2:OFF_L2 + L], in_=l2_sb)

        states = {}
        for t in range(nt):
            if t == 0:
                d = pre_in
            elif t == 1:
                d = pre_in1
            elif t == 2:
                d = load_inputs(t, it_tile=it2_pre)
            else:
                d = load_inputs(t)
            tt = pre_tr if t == 0 else make_trans(t, d)
            states[t] = stage1(t, d, tt)
            if t >= 1:
                tail(t - 1, states.pop(t - 1))
        tail(nt - 1, states.pop(nt - 1))

    nc.compile()
    return nc


_NC_CACHE = []


def _get_program():
    if not _NC_CACHE:
        _NC_CACHE.append(_build_program())
    return _NC_CACHE[0]


def _prep_in_maps(inputs):
    f32c = lambda a: np.ascontiguousarray(np.asarray(a), dtype=np.float32)
    tr = lambda a: np.ascontiguousarray(np.asarray(a, dtype=np.float32).T)
    shard = {
        "it": f32c(inputs["I_t"]).reshape(N_CORES, BL, D),
        "h": f32c(inputs["h"]).reshape(N_CORES, BL, H),
        "h2": f32c(inputs["h2"]).reshape(N_CORES, BL, H),
        "spp": f32c(inputs["sigma_p_prev"]).reshape(N_CORES, BL, L),
        "tffp": f32c(inputs["theta_ff_prev"]).reshape(N_CORES, BL, L),
        "tp": f32c(inputs["theta_prev"]).reshape(N_CORES, BL, L),
        "sstp": f32c(inputs["sst_inh_prev"]).reshape(N_CORES, BL, L),
        "epsz": f32c(inputs["eps_z"]).reshape(N_CORES, BL, L),
        "epszh": f32c(inputs["eps_zhat"]).reshape(N_CORES, BL, L),
    }
    rep = {
        "wpm_t": tr(inputs["W_post_mu"]),
        "wps_t": tr(inputs["W_post_sigma"]),
        "wzh_t": tr(inputs["W_z_to_h"]),
        "whh_t": tr(inputs["W_h_to_h"]),
        "wh2h2_t": tr(inputs["W_h2_to_h2"]),
        "wzh2_t": tr(inputs["W_z_to_h2"]),
        "wprm_t": tr(inputs["W_prior_mu"]),
        "wprs_t": tr(inputs["W_prior_sigma"]),
        "wvip_t": tr(inputs["W_vip"]),
        "wt2z_t": tr(inputs["W_theta_to_z"]),
        "wi2t_t": tr(inputs["W_I_to_theta"]),
        "wrec1": f32c(inputs["W_rec1"]),
        "wrec2_t": tr(inputs["W_rec2"]),
        "bps": f32c(inputs["b_prior_sigma"]).reshape(1, L),
    }
    return [
        {**{k: v[i] for k, v in shard.items()}, **rep} for i in range(N_CORES)
    ]


def run(inputs, trace=False, **kw):
    nc = _get_program()
    in_maps = _prep_in_maps(inputs)
    res = run_bass_kernel_spmd(
        nc, in_maps, core_ids=list(range(N_CORES)), trace=trace, **kw
    )
    out = np.concatenate([res.results[i]["out"] for i in range(N_CORES)], axis=0)
    return out, res


def kernel(**inputs):
    out, _ = run(inputs)
    return out

